# revision 35
# baseline (speedup 1.0000x reference)
"""Multi-head attention (B=4, S=2048, D=512, H=8) on 8 trn2 cores.

Sharding: core c handles batch b=c//2 and the head-quad qh=c%2 (heads
4*qh..4*qh+3, i.e. 2 head-PAIRS). The kernel is restructured around keeping
the Scalar (ACT) engine -- which does the softmax exp and is the true
bottleneck at 1 elem/cycle/lane @1.2GHz (~147us/core) -- saturated:

- Scores for a head PAIR run concurrently on the PE via row tiling
  (tile_position (0,0)/(64,0), K=64 each): both heads' scores for one
  128-key chunk land in one [128, 1024] psum tile in ~512 cycles, and a
  single N=1024 exp call covers the pair.
- Units are (query-block of 512, pair); sp is double-buffered so the PE
  writes scores for chunk j+1 while ACT exps chunk j; attn@v (with the
  ones-column denominator trick) drains at bf16 from SBUF behind exp.
- All projection / output-projection work is interleaved into the PE's
  slack inside the jc loops (useful filler instead of the old HAM-bridge
  dummies), and per-chunk kT/qT/vv tiles let the first exp start ~8us in.

All matmuls in float32r (1 cycle/row). Softmax skips max-subtraction
(|s| < ~55 whp, exp fits fp32/bf16) exactly like the reference within tol.
"""
import sys

sys.path.insert(0, "/opt/trn_rl_repo")
import numpy as np
import ml_dtypes

B, S, D, H, HD = 4, 2048, 512, 8, 64
HPC = 4          # heads per core
DQ = HPC * HD    # 256 projection dims per core
NCORES = 8
VW = HD + 1      # v block width incl. ones column (65)
QB = 512         # query block
NQB = S // QB    # 4
NJC = S // 128   # 16 key chunks

_cache = {}


def _build_nc():
    import concourse.bacc as bacc
    import concourse.mybir as mybir
    import concourse.tile as tile

    F32, F32R = mybir.dt.float32, mybir.dt.float32r
    BF16, FP16 = mybir.dt.bfloat16, mybir.dt.float16
    EXP = mybir.ActivationFunctionType.Exp

    nc = bacc.Bacc("TRN2", target_bir_lowering=False, debug=False)

    # xTd[d] = x[b].T[128d:128(d+1), :], row-contiguous so each input DMA
    # reads 2KB+ sequential DRAM rows (single FIFO HW queue, few descriptors).
    xTd = nc.dram_tensor("xTd", [4, 128, S], FP16, kind="ExternalInput")
    wqT = nc.dram_tensor("wqT", [D, DQ], FP16, kind="ExternalInput")
    wkT = nc.dram_tensor("wkT", [D, DQ], FP16, kind="ExternalInput")
    wvT = nc.dram_tensor("wvT", [D, DQ], FP16, kind="ExternalInput")
    woT = nc.dram_tensor("woT", [DQ, D], BF16, kind="ExternalInput")
    # outTc[m, qb] = outT[m*128:(m+1)*128, qb*512:(qb+1)*512] (host unpacks)
    outTc = nc.dram_tensor("outTc", [4, NQB, 128, 512], BF16,
                           kind="ExternalOutput")
    identT = nc.dram_tensor("identT", [128, 128], F32R, kind="ExternalInput")

    with tile.TileContext(nc) as tc:
        with tc.tile_pool(name="sb", bufs=1) as sb:
            psum = tc.tile_pool(name="psum", bufs=1, space="PSUM")
            pp = psum.__enter__()

            # ---- input DMAs, priority order for earliest first-exp:
            # wk/wq pair-0 column slices -> x chunk 0 -> wv -> x chunk 1 ->
            # pair-1 weight slices -> x chunks 2,3 -> wo.  (Single HW DMA
            # queue is FIFO, so emission order = landing order.)
            wk = [sb.tile([128, DQ], FP16, tag=f"wk{d}", name=f"wk{d}")
                  for d in range(4)]
            wq = [sb.tile([128, DQ], FP16, tag=f"wq{d}", name=f"wq{d}")
                  for d in range(4)]
            xt = [sb.tile([128, S], FP16, tag=f"xt{d}", name=f"xt{d}")
                  for d in range(4)]
            # Inputs split across BOTH HWDGE queues (Sync + Scalar) for ~2x
            # aggregate DMA rate.  Input loads have no wait-sems, so the DMA
            # enqueues on the Scalar queue retire immediately and do not
            # delay the later ACTIVATE stream.
            wv = [sb.tile([128, DQ], FP16, tag=f"wv{d}", name=f"wv{d}")
                  for d in range(4)]
            wo = [sb.tile([64, D], BF16, tag=f"wo{kc}", name=f"wo{kc}")
                  for kc in range(4)]
            ident = sb.tile([128, 128], F32R, tag="ident", name="ident")
            for d in range(4):  # sync: wk-p0
                nc.sync.dma_start(out=wk[d][:, 0:128],
                                  in_=wkT[128 * d:128 * (d + 1), 0:128])
            for d in range(4):  # scalar: wq-p0
                nc.scalar.dma_start(out=wq[d][:, 0:128],
                                    in_=wqT[128 * d:128 * (d + 1), 0:128])
            for d in range(4):
                nc.sync.dma_start(out=xt[d][:, 0:512], in_=xTd[d, :, 0:512])
                nc.scalar.dma_start(out=xt[d][:, 512:1024],
                                    in_=xTd[d, :, 512:1024])
            for d in range(4):  # sync: wv
                nc.sync.dma_start(out=wv[d][:], in_=wvT[128 * d:128 * (d + 1), :])
            for lst, dram in ((wk, wkT), (wq, wqT)):  # scalar: pair-1 slices
                for d in range(4):
                    nc.scalar.dma_start(out=lst[d][:, 128:256],
                                        in_=dram[128 * d:128 * (d + 1), 128:256])
            for d in range(4):
                nc.sync.dma_start(out=xt[d][:, 1024:1536],
                                  in_=xTd[d, :, 1024:1536])
                nc.scalar.dma_start(out=xt[d][:, 1536:2048],
                                    in_=xTd[d, :, 1536:2048])
            for kc in range(4):
                nc.sync.dma_start(out=wo[kc][:],
                                  in_=woT[64 * kc:64 * (kc + 1), :])
            nc.scalar.dma_start(out=ident[:], in_=identT[:, :])

            # ---- ACT table pre-load: tiny exp at t0 (hidden under DMA) ----
            dumm = sb.tile([128, 8], F32, tag="dumm", name="dumm")
            nc.vector.memset(dumm[:], 0.0)
            dumo = sb.tile([128, 8], F32, tag="dumo", name="dumo")
            nc.scalar.activation(dumo[:], dumm[:], EXP)

            # ---- persistent tiles ----
            # kT/qT per (pair, chunk): partitions 0-63 = head 2p, 64-127 = 2p+1
            kTt = [[sb.tile([128, 512], FP16, tag=f"kT{p}_{sc}", name=f"kT{p}_{sc}")
                    for sc in range(4)] for p in range(2)]
            qTt = [[sb.tile([128, 512], FP16, tag=f"qT{p}_{qb}", name=f"qT{p}_{qb}")
                    for qb in range(NQB)] for p in range(2)]
            vvt = [sb.tile([128, HPC * VW], BF16, tag=f"vv{jc}", name=f"vv{jc}")
                   for jc in range(NJC)]
            oTn = [[sb.tile([64, QB], BF16, tag=f"oTn{h}_{qb}", name=f"oTn{h}_{qb}")
                    for qb in range(NQB)] for h in range(HPC)]
            ones32 = sb.tile([128, 1], F32, tag="ones32", name="ones32")
            nc.vector.memset(ones32[:], 1.0)
            ones32r = sb.tile([128, 1], F32R, tag="ones32r", name="ones32r")
            nc.vector.tensor_copy(out=ones32r[:], in_=ones32[:])
            # [1, 2] f32r ones: fp32r matmuls need an even innermost N, so
            # the epilogue sum-transpose writes column PAIRS.
            ones2r = sb.tile([128, 2], F32R, tag="ones2r", name="ones2r")
            nc.vector.tensor_copy(
                out=ones2r[:], in_=ones32[:].to_broadcast((128, 2)))

            # ---- proj group emitters (each ~1-2k PE cycles + a DVE copy) ----
            def kq_group(dst, wsb, p, col0):
                # dst[:, :] = (w pair-slice).T @ x[:, col0:col0+512]
                ps = pp.tile([128, 512], F32, tag="pg", bufs=2, name="ps")
                for d in range(4):
                    nc.tensor.matmul(
                        ps[:], wsb[d][:, p * 128:(p + 1) * 128],
                        xt[d][:, col0:col0 + 512],
                        start=(d == 0), stop=(d == 3))
                nc.vector.tensor_copy(out=dst[:], in_=ps[:])

            def vv_group(jc):
                psv = pp.tile([128, 512], F32, tag="pg", bufs=2, name="psv")
                for d in range(4):
                    nc.tensor.matmul(
                        psv[:, 0:DQ], xt[d][:, jc * 128:(jc + 1) * 128],
                        wv[d][:, :], start=(d == 0), stop=(d == 3))
                vv_ones = vvt[jc][:, :].rearrange(
                    "p (g w) -> p g w", w=VW)[:, :, HD:HD + 1]
                nc.vector.tensor_copy(
                    out=vv_ones, in_=ones32[:].to_broadcast((128, HPC, 1)))
                for h in range(HPC):
                    nc.vector.tensor_copy(
                        out=vvt[jc][:, VW * h: VW * h + HD],
                        in_=psv[:, HD * h: HD * (h + 1)])

            def outproj_group(m, qb):
                # outT[m-chunk, qb-block] = sum_h woT[h-dims, m-chunk].T @ oTn
                po = pp.tile([128, 512], F32, tag="pg", bufs=2, name="po")
                for kc in range(4):
                    nc.tensor.matmul(
                        po[:], wo[kc][:, m * 128:(m + 1) * 128],
                        oTn[kc][qb][:], start=(kc == 0), stop=(kc == 3))
                ob = sb.tile([128, 512], BF16, tag="ob", bufs=4, name="ob")
                nc.vector.tensor_copy(out=ob[:], in_=po[:])
                nc.sync.dma_start(out=outTc[m, qb], in_=ob[:])

            # ---- explicit filler schedule: unit u = qb*2 + p; slots[u][jc]
            # is a list of closures emitted right after attnv(jc-1), i.e. in
            # the PE's ACT-bound slack.  Placement respects (a) emission
            # before consumption, (b) input-DMA landing times (so a stalled
            # filler never blocks the FIFO PE queue ahead of scores).
            slots = [dict() for _ in range(2 * NQB)]

            def put(u, jc, fn):
                slots[u].setdefault(jc, []).append(fn)

            # unit 0 = (qb0, p0): stream in remaining vv + kT chunks
            for k in range(2, NJC):
                put(0, k - 2, lambda jc=k: vv_group(jc))
            put(0, 3, lambda: kq_group(kTt[0][1], wk, 0, 512))
            put(0, 5, lambda: kq_group(kTt[0][2], wk, 0, 1024))
            put(0, 9, lambda: kq_group(kTt[0][3], wk, 0, 1536))
            put(0, 11, lambda: kq_group(kTt[1][0], wk, 1, 0))
            put(0, 13, lambda: kq_group(kTt[1][1], wk, 1, 512))
            put(0, 15, lambda: kq_group(qTt[1][0], wq, 1, 0))
            # unit 1 = (qb0, p1): its own late kT chunks + next qT
            put(1, 1, lambda: kq_group(kTt[1][2], wk, 1, 1024))
            put(1, 3, lambda: kq_group(kTt[1][3], wk, 1, 1536))
            put(1, 9, lambda: kq_group(qTt[0][1], wq, 0, 512))
            for qb in range(1, NQB):
                u0 = 2 * qb
                # outproj of the previous qblock goes in the ODD unit (a
                # full unit, ~18us, after the epilogues that produce its
                # oTn inputs -- their DMA round-trip chain must not stall
                # the PE FIFO in front of scores).
                for m, (du, j) in enumerate(((0, 11), (0, 13), (1, 5), (1, 7))):
                    put(u0 + du, j, lambda m=m, qb=qb: outproj_group(m, qb - 1))
                put(u0, 9, lambda qb=qb: kq_group(qTt[1][qb], wq, 1, qb * 512))
                if qb + 1 < NQB:
                    put(u0 + 1, 9, lambda qb=qb: kq_group(
                        qTt[0][qb + 1], wq, 0, (qb + 1) * 512))

            # ---- phase P: minimal prefix before unit (qb0, p0) ----
            with nc.named_scope("proj"):
                # HAM warm-up: the ~14us input-DMA window would otherwise
                # leave the PE clock-gated at 1.2GHz when the first
                # projections land.  Dependency-free fp32 matmuls keep the
                # activity monitor busy right up to the first kq group.
                wu = sb.tile([128, 512], F32, tag="wu", name="wu")
                nc.vector.memset(wu[:], 0.5)
                wups = pp.tile([128, 1024], F32, tag="sp", bufs=2, name="wups")
                for _ in range(4):
                    nc.tensor.matmul(
                        wups[:, 0:512], wu[:, 0:128], wu[:],
                        start=True, stop=True, skip_group_check=True)
                kq_group(kTt[0][0], wk, 0, 0)
                kq_group(qTt[0][0], wq, 0, 0)
                vv_group(0)
                vv_group(1)

            # ---- attention epilogue: softmax denominators, all on-chip.
            # otu row 64 holds sums[q].  Stage A transposes it to [128, 4]
            # via 4 tiny matmuls and takes the reciprocal on the DVE (the
            # [128, p] layout gives it lanes).  Stage B broadcasts it back to
            # [64, 512] in one pass: lhsT = recT column broadcast (stride-0)
            # against the identity, so psC[d, 128j+n] = 1/sums[128j+n].
            def epi_A(otu, recT):
                psA = pp.tile([128, 512], F32, tag="pg", bufs=2, name="psA")
                for j in range(4):
                    nc.tensor.matmul(
                        psA[:, 2 * j:2 * j + 2],
                        otu[64:65, 128 * j:128 * (j + 1)],
                        ones2r[64:65, :], start=True, stop=True)
                with nc.allow_low_precision(reason="f32r keeps full fp32 bits"):
                    nc.vector.reciprocal(recT[:], psA[:, 0:8:2])

            def epi_B(h, qb, otu, recT):
                psC = pp.tile([128, 512], F32, tag="pg", bufs=2, name="psC")
                for j in range(4):
                    nc.tensor.matmul(
                        psC[0:64, 128 * j:128 * (j + 1)],
                        recT[:, j:j + 1].to_broadcast((128, 64)),
                        ident[:, :], start=True, stop=True)
                nc.vector.tensor_mul(
                    out=oTn[h][qb][:], in0=otu[0:64, :], in1=psC[0:64, :])

            sb.tile([128, 29696], BF16, tag="pad", name="pad")
            with nc.named_scope("attn"):
                # single software-pipelined stream over g = u*16 + jc:
                # attnv lags scores/exp by one g GLOBALLY (across unit
                # boundaries too) so the FIFO PE queue never parks behind
                # an exp-dependent matmul and never idles a HAM window.
                ats = {}
                ops = {}

                def attnv_g(g):
                    u, jc = divmod(g, 16)
                    p = u % 2
                    op0, op1 = ops[u]
                    for e, op_t in ((0, op0), (1, op1)):
                        nc.tensor.matmul(
                            op_t[:],
                            vvt[jc][:, VW * (2 * p + e): VW * (2 * p + e) + VW],
                            ats[g][:, e * 512:(e + 1) * 512],
                            start=(jc == 0), stop=(jc == 15))

                def epi_drain(u):
                    # op psum -> otu (DVE), then PE stages as slot fillers
                    qb, p = u // 2, u % 2
                    epis = []
                    for e, op_t in ((0, ops[u][0]), (1, ops[u][1])):
                        otu = sb.tile([65, QB], F32R, tag="otu", bufs=2,
                                      name="otu")
                        nc.vector.tensor_copy(out=otu[:], in_=op_t[:])
                        recT = sb.tile([128, 4], F32R, tag="recT", bufs=2,
                                       name="recT")
                        epis.append((2 * p + e, otu, recT))
                    return epis

                for g in range(2 * NQB * 16):
                    u, jc = divmod(g, 16)
                    qb, p = u // 2, u % 2
                    if jc == 0:
                        ops[u] = (
                            pp.tile([65, QB], F32, tag="op0", bufs=1, name="op0"),
                            pp.tile([65, QB], F32, tag="op1", bufs=1, name="op1"))
                    sp = pp.tile([128, 1024], F32, tag="sp", bufs=2, name="sp")
                    sc, c0 = jc // 4, (jc % 4) * 128
                    for e in range(2):
                        nc.tensor.matmul(
                            sp[:, e * 512:(e + 1) * 512],
                            kTt[p][sc][64 * e:64 * e + 64, c0:c0 + 128],
                            qTt[p][qb][64 * e:64 * e + 64, :],
                            start=True, stop=True,
                            tile_position=(64 * e, 0))
                    at = sb.tile([128, 1024], BF16, tag="at", bufs=4, name="at")
                    nc.scalar.activation(at[:], sp[:], EXP)
                    ats[g] = at
                    if g > 0:
                        attnv_g(g - 1)
                    if jc == 0 and u > 0:
                        qbp = (u - 1) // 2
                        for e, (h, otu, recT) in enumerate(epi_drain(u - 1)):
                            put(u, 1 + e, lambda otu=otu, recT=recT:
                                epi_A(otu, recT))
                            put(u, 3 + e, lambda h=h, qbp=qbp, otu=otu,
                                recT=recT: epi_B(h, qbp, otu, recT))
                    for fn in slots[u].get(jc, ()):
                        fn()
                attnv_g(2 * NQB * 16 - 1)
                for h, otu, recT in epi_drain(2 * NQB - 1):
                    epi_A(otu, recT)
                    epi_B(h, NQB - 1, otu, recT)

            with nc.named_scope("outproj"):
                for m in range(4):
                    outproj_group(m, NQB - 1)

            psum.__exit__(None, None, None)

    nc.compile()
    return nc


def _get_nc():
    if "nc" not in _cache:
        _cache["nc"] = _build_nc()
    return _cache["nc"]


def _in_maps(x, w_qkv, w_out):
    x = np.asarray(x, dtype=np.float32)
    w_qkv = np.asarray(w_qkv, dtype=np.float32)
    w_out = np.asarray(w_out, dtype=np.float32)
    maps = []
    for c in range(NCORES):
        b, qh = c // 2, c % 2
        r0 = qh * DQ
        xT = x[b].T  # [D, S]
        xTd = np.ascontiguousarray(xT).astype(np.float16).reshape(4, 128, S)
        maps.append({
            "xTd": xTd,
            "identT": np.eye(128, dtype=np.float32),
            "wqT": np.ascontiguousarray(
                w_qkv[r0:r0 + DQ].T).astype(np.float16),
            "wkT": np.ascontiguousarray(
                w_qkv[D + r0:D + r0 + DQ].T).astype(np.float16),
            "wvT": np.ascontiguousarray(
                w_qkv[2 * D + r0:2 * D + r0 + DQ].T).astype(np.float16),
            "woT": np.ascontiguousarray(
                w_out[:, r0:r0 + DQ].T).astype(ml_dtypes.bfloat16),
        })
    return maps


def _gather(results):
    out = np.empty((B, S, D), np.float32)
    for b in range(B):
        acc = (results[2 * b]["outTc"].astype(np.float32)
               + results[2 * b + 1]["outTc"].astype(np.float32))
        # [4(m), NQB, 128, 512] -> outT [D, S] -> out [S, D]
        outT = acc.transpose(0, 2, 1, 3).reshape(D, S)
        out[b] = outT.T
    return out


def run(x, w_qkv, w_out, trace=False):
    from concourse.bass_utils import run_bass_kernel_spmd

    nc = _get_nc()
    res = run_bass_kernel_spmd(
        nc, _in_maps(x, w_qkv, w_out), core_ids=list(range(NCORES)), trace=trace,
    )
    return _gather(res.results), res


def kernel(x, w_qkv, w_out):
    out, _ = run(x, w_qkv, w_out)
    return out


# revision 36
# speedup vs baseline: 1.1894x; 1.1894x over previous
"""Multi-head attention (B=4, S=2048, D=512, H=8) on 8 trn2 cores.

Sharding: core c handles batch b=c//2 and the head-quad qh=c%2 (heads
4*qh..4*qh+3, i.e. 2 head-PAIRS). The kernel is restructured around keeping
the Scalar (ACT) engine -- which does the softmax exp and is the true
bottleneck at 1 elem/cycle/lane @1.2GHz (~147us/core) -- saturated:

- Scores for a head PAIR run concurrently on the PE via row tiling
  (tile_position (0,0)/(64,0), K=64 each): both heads' scores for one
  128-key chunk land in one [128, 1024] psum tile in ~512 cycles, and a
  single N=1024 exp call covers the pair.
- Units are (query-block of 512, pair); sp is double-buffered so the PE
  writes scores for chunk j+1 while ACT exps chunk j; attn@v (with the
  ones-column denominator trick) drains at bf16 from SBUF behind exp.
- All projection / output-projection work is interleaved into the PE's
  slack inside the jc loops (useful filler instead of the old HAM-bridge
  dummies), and per-chunk kT/qT/vv tiles let the first exp start ~8us in.

All matmuls in float32r (1 cycle/row). Softmax skips max-subtraction
(|s| < ~55 whp, exp fits fp32/bf16) exactly like the reference within tol.
"""
import sys

sys.path.insert(0, "/opt/trn_rl_repo")
import numpy as np
import ml_dtypes

B, S, D, H, HD = 4, 2048, 512, 8, 64
HPC = 4          # heads per core
DQ = HPC * HD    # 256 projection dims per core
NCORES = 8
VW = HD + 1      # v block width incl. ones column (65)
QB = 512         # query block
NQB = S // QB    # 4
NJC = S // 128   # 16 key chunks

_cache = {}


def _build_nc():
    import concourse.bacc as bacc
    import concourse.mybir as mybir
    import concourse.tile as tile

    F32, F32R = mybir.dt.float32, mybir.dt.float32r
    BF16, FP16 = mybir.dt.bfloat16, mybir.dt.float16
    EXP = mybir.ActivationFunctionType.Exp

    nc = bacc.Bacc("TRN2", target_bir_lowering=False, debug=False)

    # xTd[d] = x[b].T[128d:128(d+1), :], row-contiguous so each input DMA
    # reads 2KB+ sequential DRAM rows (single FIFO HW queue, few descriptors).
    xTd = nc.dram_tensor("xTd", [4, 128, S], FP16, kind="ExternalInput")
    wqT = nc.dram_tensor("wqT", [D, DQ], FP16, kind="ExternalInput")
    wkT = nc.dram_tensor("wkT", [D, DQ], FP16, kind="ExternalInput")
    wvT = nc.dram_tensor("wvT", [D, DQ], FP16, kind="ExternalInput")
    woT = nc.dram_tensor("woT", [DQ, D], BF16, kind="ExternalInput")
    # outTc[m, qb] = outT[m*128:(m+1)*128, qb*512:(qb+1)*512] (host unpacks)
    outTc = nc.dram_tensor("outTc", [4, NQB, 128, 512], BF16,
                           kind="ExternalOutput")
    identT = nc.dram_tensor("identT", [128, 128], F32R, kind="ExternalInput")

    with tile.TileContext(nc) as tc:
        with tc.tile_pool(name="sb", bufs=1) as sb:
            psum = tc.tile_pool(name="psum", bufs=1, space="PSUM")
            pp = psum.__enter__()

            # ---- input DMAs, priority order for earliest first-exp:
            # wk/wq pair-0 column slices -> x chunk 0 -> wv -> x chunk 1 ->
            # pair-1 weight slices -> x chunks 2,3 -> wo.  (Single HW DMA
            # queue is FIFO, so emission order = landing order.)
            wk = [sb.tile([128, DQ], FP16, tag=f"wk{d}", name=f"wk{d}")
                  for d in range(4)]
            wq = [sb.tile([128, DQ], FP16, tag=f"wq{d}", name=f"wq{d}")
                  for d in range(4)]
            xt = [sb.tile([128, S], FP16, tag=f"xt{d}", name=f"xt{d}")
                  for d in range(4)]
            # Inputs split across BOTH HWDGE queues (Sync + Scalar) for ~2x
            # aggregate DMA rate.  Input loads have no wait-sems, so the DMA
            # enqueues on the Scalar queue retire immediately and do not
            # delay the later ACTIVATE stream.
            wv = [sb.tile([128, DQ], FP16, tag=f"wv{d}", name=f"wv{d}")
                  for d in range(4)]
            wo = [sb.tile([64, D], BF16, tag=f"wo{kc}", name=f"wo{kc}")
                  for kc in range(4)]
            ident = sb.tile([128, 128], F32R, tag="ident", name="ident")
            for d in range(4):
                nc.sync.dma_start(out=wk[d][:, 0:128],
                                  in_=wkT[128 * d:128 * (d + 1), 0:128])
                nc.gpsimd.dma_start(out=xt[d][:, 512:1024],
                                    in_=xTd[d, :, 512:1024])
            for d in range(4):
                nc.sync.dma_start(out=wq[d][:, 0:128],
                                  in_=wqT[128 * d:128 * (d + 1), 0:128])
            for d in range(4):
                nc.sync.dma_start(out=xt[d][:, 0:512], in_=xTd[d, :, 0:512])
            for lst, dram in ((wk, wkT), (wq, wqT)):  # gpsimd: pair-1 slices
                for d in range(4):
                    nc.gpsimd.dma_start(out=lst[d][:, 128:256],
                                        in_=dram[128 * d:128 * (d + 1), 128:256])
            for d in range(4):  # sync: wv
                nc.sync.dma_start(out=wv[d][:], in_=wvT[128 * d:128 * (d + 1), :])
            for d in range(4):
                nc.gpsimd.dma_start(out=xt[d][:, 1536:2048],
                                    in_=xTd[d, :, 1536:2048])
                nc.sync.dma_start(out=xt[d][:, 1024:1536],
                                  in_=xTd[d, :, 1024:1536])
            for kc in range(4):
                nc.sync.dma_start(out=wo[kc][:],
                                  in_=woT[64 * kc:64 * (kc + 1), :])
            nc.gpsimd.dma_start(out=ident[:], in_=identT[:, :])

            # ---- ACT table pre-load: tiny exp at t0 (hidden under DMA) ----
            dumm = sb.tile([128, 8], F32, tag="dumm", name="dumm")
            nc.vector.memset(dumm[:], 0.0)
            dumo = sb.tile([128, 8], F32, tag="dumo", name="dumo")
            nc.scalar.activation(dumo[:], dumm[:], EXP)

            # ---- persistent tiles ----
            # kT/qT per (pair, chunk): partitions 0-63 = head 2p, 64-127 = 2p+1
            kTt = [[sb.tile([128, 512], FP16, tag=f"kT{p}_{sc}", name=f"kT{p}_{sc}")
                    for sc in range(4)] for p in range(2)]
            qTt = [[sb.tile([128, 512], FP16, tag=f"qT{p}_{qb}", name=f"qT{p}_{qb}")
                    for qb in range(NQB)] for p in range(2)]
            vvt = [sb.tile([128, HPC * VW], BF16, tag=f"vv{jc}", name=f"vv{jc}")
                   for jc in range(NJC)]
            oTn = [[sb.tile([64, QB], BF16, tag=f"oTn{h}_{qb}", name=f"oTn{h}_{qb}")
                    for qb in range(NQB)] for h in range(HPC)]
            ones32 = sb.tile([128, 1], F32, tag="ones32", name="ones32")
            nc.vector.memset(ones32[:], 1.0)
            ones32r = sb.tile([128, 1], F32R, tag="ones32r", name="ones32r")
            nc.vector.tensor_copy(out=ones32r[:], in_=ones32[:])
            # [1, 2] f32r ones: fp32r matmuls need an even innermost N, so
            # the epilogue sum-transpose writes column PAIRS.
            ones2r = sb.tile([128, 2], F32R, tag="ones2r", name="ones2r")
            nc.vector.tensor_copy(
                out=ones2r[:], in_=ones32[:].to_broadcast((128, 2)))

            # ---- proj group emitters (each ~1-2k PE cycles + a DVE copy) ----
            def kq_group(dst, wsb, p, col0):
                # dst[:, :] = (w pair-slice).T @ x[:, col0:col0+512]
                ps = pp.tile([128, 512], F32, tag="pg", bufs=2, name="ps")
                for d in range(4):
                    nc.tensor.matmul(
                        ps[:], wsb[d][:, p * 128:(p + 1) * 128],
                        xt[d][:, col0:col0 + 512],
                        start=(d == 0), stop=(d == 3))
                nc.vector.tensor_copy(out=dst[:], in_=ps[:])

            def vv_group(jc):
                psv = pp.tile([128, 512], F32, tag="pg", bufs=2, name="psv")
                for d in range(4):
                    nc.tensor.matmul(
                        psv[:, 0:DQ], xt[d][:, jc * 128:(jc + 1) * 128],
                        wv[d][:, :], start=(d == 0), stop=(d == 3))
                vv_ones = vvt[jc][:, :].rearrange(
                    "p (g w) -> p g w", w=VW)[:, :, HD:HD + 1]
                nc.vector.tensor_copy(
                    out=vv_ones, in_=ones32[:].to_broadcast((128, HPC, 1)))
                for h in range(HPC):
                    nc.vector.tensor_copy(
                        out=vvt[jc][:, VW * h: VW * h + HD],
                        in_=psv[:, HD * h: HD * (h + 1)])

            def outproj_group(m, qb):
                # outT[m-chunk, qb-block] = sum_h woT[h-dims, m-chunk].T @ oTn
                po = pp.tile([128, 512], F32, tag="pg", bufs=2, name="po")
                for kc in range(4):
                    nc.tensor.matmul(
                        po[:], wo[kc][:, m * 128:(m + 1) * 128],
                        oTn[kc][qb][:], start=(kc == 0), stop=(kc == 3))
                ob = sb.tile([128, 512], BF16, tag="ob", bufs=4, name="ob")
                nc.vector.tensor_copy(out=ob[:], in_=po[:])
                nc.sync.dma_start(out=outTc[m, qb], in_=ob[:])

            # ---- explicit filler schedule: unit u = qb*2 + p; slots[u][jc]
            # is a list of closures emitted right after attnv(jc-1), i.e. in
            # the PE's ACT-bound slack.  Placement respects (a) emission
            # before consumption, (b) input-DMA landing times (so a stalled
            # filler never blocks the FIFO PE queue ahead of scores).
            slots = [dict() for _ in range(2 * NQB)]

            def put(u, jc, fn):
                slots[u].setdefault(jc, []).append(fn)

            # unit 0 = (qb0, p0): stream in remaining vv + kT chunks
            for k in range(2, NJC):
                put(0, k - 2, lambda jc=k: vv_group(jc))
            put(0, 3, lambda: kq_group(kTt[0][1], wk, 0, 512))
            put(0, 5, lambda: kq_group(kTt[0][2], wk, 0, 1024))
            put(0, 9, lambda: kq_group(kTt[0][3], wk, 0, 1536))
            put(0, 11, lambda: kq_group(kTt[1][0], wk, 1, 0))
            put(0, 13, lambda: kq_group(kTt[1][1], wk, 1, 512))
            put(0, 15, lambda: kq_group(qTt[1][0], wq, 1, 0))
            # unit 1 = (qb0, p1): its own late kT chunks + next qT
            put(1, 1, lambda: kq_group(kTt[1][2], wk, 1, 1024))
            put(1, 3, lambda: kq_group(kTt[1][3], wk, 1, 1536))
            put(1, 9, lambda: kq_group(qTt[0][1], wq, 0, 512))
            for qb in range(1, NQB):
                u0 = 2 * qb
                # outproj of the previous qblock goes in the ODD unit (a
                # full unit, ~18us, after the epilogues that produce its
                # oTn inputs -- their DMA round-trip chain must not stall
                # the PE FIFO in front of scores).
                for m, (du, j) in enumerate(((0, 11), (0, 13), (1, 5), (1, 7))):
                    put(u0 + du, j, lambda m=m, qb=qb: outproj_group(m, qb - 1))
                put(u0, 9, lambda qb=qb: kq_group(qTt[1][qb], wq, 1, qb * 512))
                if qb + 1 < NQB:
                    put(u0 + 1, 9, lambda qb=qb: kq_group(
                        qTt[0][qb + 1], wq, 0, (qb + 1) * 512))

            # ---- phase P: minimal prefix before unit (qb0, p0) ----
            with nc.named_scope("proj"):
                # HAM warm-up: the ~14us input-DMA window would otherwise
                # leave the PE clock-gated at 1.2GHz when the first
                # projections land.  Dependency-free fp32 matmuls keep the
                # activity monitor busy right up to the first kq group.
                wu = sb.tile([128, 512], F32, tag="wu", name="wu")
                nc.vector.memset(wu[:], 0.5)
                wups = pp.tile([128, 1024], F32, tag="sp", bufs=2, name="wups")
                for _ in range(4):
                    nc.tensor.matmul(
                        wups[:, 0:512], wu[:, 0:128], wu[:],
                        start=True, stop=True, skip_group_check=True)
                kq_group(kTt[0][0], wk, 0, 0)
                kq_group(qTt[0][0], wq, 0, 0)
                vv_group(0)
                vv_group(1)

            # ---- attention epilogue: softmax denominators, all on-chip.
            # otu row 64 holds sums[q].  Stage A transposes it to [128, 4]
            # via 4 tiny matmuls and takes the reciprocal on the DVE (the
            # [128, p] layout gives it lanes).  Stage B broadcasts it back to
            # [64, 512] in one pass: lhsT = recT column broadcast (stride-0)
            # against the identity, so psC[d, 128j+n] = 1/sums[128j+n].
            def epi_A(otu, recT):
                psA = pp.tile([128, 512], F32, tag="pg", bufs=2, name="psA")
                for j in range(4):
                    nc.tensor.matmul(
                        psA[:, 2 * j:2 * j + 2],
                        otu[64:65, 128 * j:128 * (j + 1)],
                        ones2r[64:65, :], start=True, stop=True)
                with nc.allow_low_precision(reason="f32r keeps full fp32 bits"):
                    nc.vector.reciprocal(recT[:], psA[:, 0:8:2])

            def epi_B(h, qb, otu, recT):
                psC = pp.tile([128, 512], F32, tag="pg", bufs=2, name="psC")
                for j in range(4):
                    nc.tensor.matmul(
                        psC[0:64, 128 * j:128 * (j + 1)],
                        recT[:, j:j + 1].to_broadcast((128, 64)),
                        ident[:, :], start=True, stop=True)
                nc.vector.tensor_mul(
                    out=oTn[h][qb][:], in0=otu[0:64, :], in1=psC[0:64, :])

            sb.tile([128, 29696], BF16, tag="pad", name="pad")
            with nc.named_scope("attn"):
                # single software-pipelined stream over g = u*16 + jc:
                # attnv lags scores/exp by one g GLOBALLY (across unit
                # boundaries too) so the FIFO PE queue never parks behind
                # an exp-dependent matmul and never idles a HAM window.
                ats = {}
                ops = {}

                def attnv_g(g):
                    u, jc = divmod(g, 16)
                    p = u % 2
                    op0, op1 = ops[u]
                    for e, op_t in ((0, op0), (1, op1)):
                        nc.tensor.matmul(
                            op_t[:],
                            vvt[jc][:, VW * (2 * p + e): VW * (2 * p + e) + VW],
                            ats[g][:, e * 512:(e + 1) * 512],
                            start=(jc == 0), stop=(jc == 15))

                def epi_drain(u):
                    # op psum -> otu (DVE), then PE stages as slot fillers
                    qb, p = u // 2, u % 2
                    epis = []
                    for e, op_t in ((0, ops[u][0]), (1, ops[u][1])):
                        otu = sb.tile([65, QB], F32R, tag="otu", bufs=2,
                                      name="otu")
                        nc.vector.tensor_copy(out=otu[:], in_=op_t[:])
                        recT = sb.tile([128, 4], F32R, tag="recT", bufs=2,
                                       name="recT")
                        epis.append((2 * p + e, otu, recT))
                    return epis

                for g in range(2 * NQB * 16):
                    u, jc = divmod(g, 16)
                    qb, p = u // 2, u % 2
                    if jc == 0:
                        ops[u] = (
                            pp.tile([65, QB], F32, tag="op0", bufs=1, name="op0"),
                            pp.tile([65, QB], F32, tag="op1", bufs=1, name="op1"))
                    sp = pp.tile([128, 1024], F32, tag="sp", bufs=2, name="sp")
                    sc, c0 = jc // 4, (jc % 4) * 128
                    for e in range(2):
                        nc.tensor.matmul(
                            sp[:, e * 512:(e + 1) * 512],
                            kTt[p][sc][64 * e:64 * e + 64, c0:c0 + 128],
                            qTt[p][qb][64 * e:64 * e + 64, :],
                            start=True, stop=True,
                            tile_position=(64 * e, 0))
                    at = sb.tile([128, 1024], BF16, tag="at", bufs=4, name="at")
                    nc.scalar.activation(at[:], sp[:], EXP)
                    ats[g] = at
                    if g > 0:
                        attnv_g(g - 1)
                    if jc == 0 and u > 0:
                        qbp = (u - 1) // 2
                        for e, (h, otu, recT) in enumerate(epi_drain(u - 1)):
                            put(u, 1 + e, lambda otu=otu, recT=recT:
                                epi_A(otu, recT))
                            put(u, 3 + e, lambda h=h, qbp=qbp, otu=otu,
                                recT=recT: epi_B(h, qbp, otu, recT))
                    for fn in slots[u].get(jc, ()):
                        fn()
                attnv_g(2 * NQB * 16 - 1)
                for h, otu, recT in epi_drain(2 * NQB - 1):
                    epi_A(otu, recT)
                    epi_B(h, NQB - 1, otu, recT)

            with nc.named_scope("outproj"):
                for m in range(4):
                    outproj_group(m, NQB - 1)

            psum.__exit__(None, None, None)

    nc.compile()
    return nc


def _get_nc():
    if "nc" not in _cache:
        _cache["nc"] = _build_nc()
    return _cache["nc"]


def _in_maps(x, w_qkv, w_out):
    x = np.asarray(x, dtype=np.float32)
    w_qkv = np.asarray(w_qkv, dtype=np.float32)
    w_out = np.asarray(w_out, dtype=np.float32)
    maps = []
    for c in range(NCORES):
        b, qh = c // 2, c % 2
        r0 = qh * DQ
        xT = x[b].T  # [D, S]
        xTd = np.ascontiguousarray(xT).astype(np.float16).reshape(4, 128, S)
        maps.append({
            "xTd": xTd,
            "identT": np.eye(128, dtype=np.float32),
            "wqT": np.ascontiguousarray(
                w_qkv[r0:r0 + DQ].T).astype(np.float16),
            "wkT": np.ascontiguousarray(
                w_qkv[D + r0:D + r0 + DQ].T).astype(np.float16),
            "wvT": np.ascontiguousarray(
                w_qkv[2 * D + r0:2 * D + r0 + DQ].T).astype(np.float16),
            "woT": np.ascontiguousarray(
                w_out[:, r0:r0 + DQ].T).astype(ml_dtypes.bfloat16),
        })
    return maps


def _gather(results):
    out = np.empty((B, S, D), np.float32)
    for b in range(B):
        acc = (results[2 * b]["outTc"].astype(np.float32)
               + results[2 * b + 1]["outTc"].astype(np.float32))
        # [4(m), NQB, 128, 512] -> outT [D, S] -> out [S, D]
        outT = acc.transpose(0, 2, 1, 3).reshape(D, S)
        out[b] = outT.T
    return out


def run(x, w_qkv, w_out, trace=False):
    from concourse.bass_utils import run_bass_kernel_spmd

    nc = _get_nc()
    res = run_bass_kernel_spmd(
        nc, _in_maps(x, w_qkv, w_out), core_ids=list(range(NCORES)), trace=trace,
    )
    return _gather(res.results), res


def kernel(x, w_qkv, w_out):
    out, _ = run(x, w_qkv, w_out)
    return out
